# revision 1
# baseline (speedup 1.0000x reference)
"""Block-diagonal grouped matmul (nn_MatrixApply) on 8 TRN2 NeuronCores.

Math: out[s:s+g] = mat_i @ x[s:s+g] for 15 consecutive sample groups.
Equivalently out = BD @ x_flat with BD = blockdiag(mat_0..mat_14) (2048x2048)
and x_flat = x.reshape(2048, 512*21).

Sharding: sequence-parallel. The free dim L*A = 10752 is split into 8
contiguous chunks of 1344 (= 64*21, so each core owns x[:, 64c:64(c+1), :]).
Mats are replicated. No collectives; host concatenates the slices.

Device kernel: BD is tiled on the regular 128-grid; only the 50 nonzero
128x128 tiles are kept. Tiles are pre-transposed and packed on the host so a
single DMA lands every lhsT ready-to-use in SBUF. Per N-chunk of 336 columns:
strided DMA in, matmuls PSUM-accumulated per output row-tile, 16 DVE
evacuations, one DMA out.

Compute modes:
  bf16x3 (default) - inputs hi/lo-split into bf16 on the host;
      out = Wh@Xh + Wh@Xl + Wl@Xh runs at full PE rate (1 cyc/row vs 4 for
      fp32) with ~4e-6 relative error. DMA bytes unchanged (2x bf16 = 1x f32).
  f32    - native fp32 matmul, ~9e-7 rel err, 4 cyc/row.
  f32r   - reduced-precision fp32 matmul mode, ~1.7e-4 rel err, 1 cyc/row.
"""

import numpy as np
import ml_dtypes

import concourse.bacc as bacc
import concourse.bass as bass
import concourse.mybir as mybir
import concourse.tile as tile
from concourse import bass_utils

BF16 = ml_dtypes.bfloat16

GROUP_SIZES = (64, 128, 256, 96, 160, 224, 192, 288, 320, 112, 80, 48, 32, 16, 32)
LENGTH = 512
ALPHABET = 21
N_SAMPLES = 2048
N_CORES = 8
FREE = LENGTH * ALPHABET            # 10752
FREE_PER_CORE = FREE // N_CORES     # 1344
P = 128
NT = N_SAMPLES // P                 # 16 row/col tiles of the 2048-sample dim
NC_CHUNK = 448                      # free-dim tile per matmul (<=512, >=256)
N_CHUNKS = FREE_PER_CORE // NC_CHUNK  # 3

MODE = "packed"


# ---- group-aligned bin/stripe structure (mode "packed") ----
# The 2048-sample dim is cut at group boundaries into bins <= 128 (big groups
# into 128-chunks + remainder, consecutive small groups merged while <= 128).
# Bins are placed into 128-partition SBUF "stripes" at 32-aligned offsets
# satisfying the matmul tile_position legality (size>64 -> off 0,
# 32<size<=64 -> off in {0,64}, size<=32 -> off in {0,32,64,96}).
# A matmul tile exists per (M-bin, K-bin) pair intersected by a group:
# 40 tiles vs 50 on the naive 128-grid.
def _make_bins():
    bins = []
    s = 0
    pend = None  # [start, size]
    for g in GROUP_SIZES:
        if g > P:
            if pend is not None:
                bins.append(tuple(pend)); pend = None
            o = 0
            while o < g:
                c = min(P, g - o)
                bins.append((s + o, c)); o += c
        elif pend is None:
            pend = [s, g]
        elif pend[1] + g <= P:
            pend[1] += g
        else:
            bins.append(tuple(pend)); pend = [s, g]
        s += g
    if pend is not None:
        bins.append(tuple(pend))
    return bins


BINS = _make_bins()          # 20 x (start, size)


def _legal_offsets(size):
    if size > 64:
        return [0]
    if size > 32:
        return [0, 64]
    return [0, 32, 64, 96]


def _place_bins():
    place = {}
    stripes = []
    for i in sorted(range(len(BINS)), key=lambda i: -BINS[i][1]):
        sz = BINS[i][1]
        done = False
        for si, st in enumerate(stripes):
            for off in _legal_offsets(sz):
                if all(off + sz <= o2 or off >= o2 + s2 for (o2, s2) in st):
                    st.append((off, sz)); place[i] = (si, off); done = True
                    break
            if done:
                break
        if not done:
            stripes.append([(0, sz)]); place[i] = (len(stripes) - 1, 0)
    used = [max(o + s for (o, s) in st) for st in stripes]
    return place, used


PLACE, STRIPE_USED = _place_bins()   # bin -> (stripe, offset); per-stripe rows used
N_STRIPES = len(STRIPE_USED)         # 17


def _make_ptiles():
    """Distinct (mbin, kbin) matmul tiles, grouped per M-stripe, full-K first."""
    s = 0
    pairs = set()
    for g in GROUP_SIZES:
        bs = [i for i, (o, z) in enumerate(BINS) if o < s + g and o + z > s]
        for mb in bs:
            for kb in bs:
                pairs.add((mb, kb))
        s += g
    # weight free-dim offsets (hi at woff, lo at woff+msize)
    woff = {}
    off = 0
    for (mb, kb) in sorted(pairs):
        woff[(mb, kb)] = off
        off += 2 * BINS[mb][1]
    # per-stripe list, K=128 tiles first (guarantees the start=True matmul
    # occupies all PE rows, so no later matmul of the same accumulation
    # group can run concurrently with the bank clear)
    per_stripe = [[] for _ in range(N_STRIPES)]
    for (mb, kb) in sorted(pairs, key=lambda p: (PLACE[p[0]][0], -BINS[p[1]][1])):
        per_stripe[PLACE[mb][0]].append((mb, kb))
    return per_stripe, woff, off


PTILES_PER_STRIPE, WOFF, W_FREE = _make_ptiles()  # W_FREE = 8192



# Emission blocks of M-bins. Bins sharing a stripe stay in one block; pairs
# are chosen so their trailing thin-K matmuls occupy disjoint 32-row groups
# of the PE array (tile_position row concurrency): tail-aligned interleave
# makes them adjacent in the PE stream, so each pair costs ~one matmul span.
BIN_BLOCKS = [[1], [2], [3], [7, 11], [14, 9], [15, 12], [16, 5],
              [4, 6], [8, 13], [0, 10], [17], [18], [19]]


def _dma_order():
    """K-bins in first-use order of the matmul blocks, so the FIFO HWDGE
    rings load what the earliest blocks need first."""
    order = []
    for block in BIN_BLOCKS:
        for mb in block:
            for (_, kb) in _tiles_of_mbin(mb):
                if kb not in order:
                    order.append(kb)
    for b in range(len(BINS)):
        if b not in order:
            order.append(b)
    return order


def _tiles_of_mbin(mb):
    """(mb, kb) tiles of M-bin mb, full-K first (start=True safety)."""
    st = PLACE[mb][0]
    return [t for t in PTILES_PER_STRIPE[st] if t[0] == mb]


DMA_ORDER = _dma_order()




def _nonzero_tiles():
    """(I, J) 128-grid tiles where blockdiag(mats) is nonzero, row-major."""
    tiles = set()
    start = 0
    for g in GROUP_SIZES:
        b0, b1 = start // P, (start + g - 1) // P
        for i in range(b0, b1 + 1):
            for j in range(b0, b1 + 1):
                tiles.add((i, j))
        start += g
    return sorted(tiles)


TILES = _nonzero_tiles()
TILE_IDX = {t: i for i, t in enumerate(TILES)}
# contributors per output row-tile I: ordered list of (J, packed tile index)
CONTRIB = [
    [(j, TILE_IDX[(i, j)]) for (i2, j) in TILES if i2 == i] for i in range(NT)
]


def _bd(mats):
    bd = np.zeros((N_SAMPLES, N_SAMPLES), dtype=np.float32)
    start = 0
    for m in mats:
        g = m.shape[0]
        bd[start:start + g, start:start + g] = m
        start += g
    return bd


def _tileT(bd, t):
    i, j = TILES[t]
    return bd[i * P:(i + 1) * P, j * P:(j + 1) * P].T


def pack_weights_f32(mats):
    """(128, len(TILES)*128) f32: slot t holds BD[I-block, J-block].T."""
    bd = _bd(mats)
    w = np.empty((P, len(TILES) * P), dtype=np.float32)
    for t in range(len(TILES)):
        w[:, t * P:(t + 1) * P] = _tileT(bd, t)
    return w


def pack_weights_bf16(mats):
    """(128, 2*len(TILES)*128) bf16: slots 2t / 2t+1 hold hi / lo of tile t."""
    bd = _bd(mats)
    w = np.empty((P, 2 * len(TILES) * P), dtype=BF16)
    for t in range(len(TILES)):
        blk = _tileT(bd, t)
        hi = blk.astype(BF16)
        lo = (blk - hi.astype(np.float32)).astype(BF16)
        w[:, (2 * t) * P:(2 * t + 1) * P] = hi
        w[:, (2 * t + 1) * P:(2 * t + 2) * P] = lo
    return w


def split_x(xf):
    """f32 (n, m) -> bf16 hi, lo."""
    hi = xf.astype(BF16)
    lo = (xf - hi.astype(np.float32)).astype(BF16)
    return hi, lo


def pack_weights_packed(mats):
    """(128, W_FREE) bf16 for mode 'packed': per (mbin,kbin) tile, the
    transposed BD block sits at partitions [k_off, k_off+ksz), free
    [woff, woff+msz) (hi) and [woff+msz, woff+2*msz) (lo)."""
    bd = _bd(mats)
    w = np.zeros((P, W_FREE), dtype=BF16)
    for per in PTILES_PER_STRIPE:
        for (mb, kb) in per:
            (ms, mz), (ks, kz) = BINS[mb], BINS[kb]
            ko = PLACE[kb][1]
            blkT = bd[ms:ms + mz, ks:ks + kz].T  # (kz, mz)
            hi = blkT.astype(BF16)
            lo = (blkT - hi.astype(np.float32)).astype(BF16)
            o = WOFF[(mb, kb)]
            w[ko:ko + kz, o:o + mz] = hi
            w[ko:ko + kz, o + mz:o + 2 * mz] = lo
    return w


def build_program(reps=1, mode=MODE):
    """Build the per-core Bass program.

    reps > 1 repeats the whole streaming kernel body (for wall-clock
    benchmarking via T(reps) differencing — no NTFF profiling under axon).
    """
    nc = bacc.Bacc("TRN2", target_bir_lowering=False, debug=False)
    f32 = mybir.dt.float32
    bf16 = mybir.dt.bfloat16
    o_d = nc.dram_tensor("out", (N_SAMPLES, FREE_PER_CORE), f32, kind="ExternalOutput")
    o_ap = o_d.ap().rearrange("(t p) n -> p t n", p=P)

    if mode == "packed":
        x2_d = nc.dram_tensor("x2", (N_SAMPLES, 2, FREE_PER_CORE), bf16, kind="ExternalInput")
        w_d = nc.dram_tensor("wpack", (P, W_FREE), bf16, kind="ExternalInput")
        with tile.TileContext(nc) as tc:
            with (
                tc.tile_pool(name="wpool", bufs=1) as wpool,
                tc.tile_pool(name="xpool", bufs=2 * N_STRIPES) as xpool,
                tc.tile_pool(name="opool", bufs=2 * N_STRIPES) as opool,
                tc.tile_pool(name="psum", bufs=8, space="PSUM") as psum_pool,
            ):
                w_sb = wpool.tile([P, W_FREE], bf16)
                nc.sync.dma_start(w_sb[:], w_d.ap())
                for _rep in range(reps):
                    for c in range(N_CHUNKS):
                        n0 = c * NC_CHUNK
                        # Per-stripe x tiles: a stripe's matmuls only wait for
                        # its own bins' DMAs, not the whole chunk's load.
                        # hi/lo interleaved in one DRAM tensor: one DMA per bin.
                        # Partial bins at nonzero offsets go to the other ring
                        # than offset-0 ones so their SBUF ports overlap.
                        xs = []
                        for st in range(N_STRIPES):
                            xt = xpool.tile([P, 2, NC_CHUNK], bf16, tag="x2")
                            xs.append(xt)
                        for i, b in enumerate(DMA_ORDER):
                            bs, bz = BINS[b]
                            st, off = PLACE[b]
                            eng = (nc.scalar if (bz < P and off > 0)
                                   else (nc.sync if i % 2 == 0 else nc.scalar))
                            eng.dma_start(
                                xs[st][off:off + bz, :, :],
                                x2_d.ap()[bs:bs + bz, :, n0:n0 + NC_CHUNK])
                        os_ = []
                        for st in range(N_STRIPES):
                            ot = opool.tile([P, NC_CHUNK], f32, tag="o")
                            os_.append(ot)
                        for block in BIN_BLOCKS:
                            block_stripes = []
                            for mb in block:
                                st = PLACE[mb][0]
                                if st not in block_stripes:
                                    block_stripes.append(st)
                            ps = {}
                            for st in block_stripes:
                                ps_tile = psum_pool.tile([P, NC_CHUNK], f32, tag="ps")
                                ps[st] = ps_tile
                            for mb in block:
                                mz = BINS[mb][1]
                                mo = PLACE[mb][1]
                                out_ps = ps[PLACE[mb][0]][mo:mo + mz, :]
                                mms = []
                                for (mb2, kb) in _tiles_of_mbin(mb):
                                    kz = BINS[kb][1]
                                    kst, ko = PLACE[kb]
                                    o = WOFF[(mb, kb)]
                                    wh = w_sb[ko:ko + kz, o:o + mz]
                                    wl = w_sb[ko:ko + kz, o + mz:o + 2 * mz]
                                    rh = xs[kst][ko:ko + kz, 0, :]
                                    rl = xs[kst][ko:ko + kz, 1, :]
                                    mms.append((wh, rh, (ko, mo)))
                                    mms.append((wh, rl, (ko, mo)))
                                    mms.append((wl, rh, (ko, mo)))
                                # one accumulation group per M-bin (start/stop
                                # clears are per-partition, so bins sharing a
                                # psum bank at disjoint partitions are safe).
                                # skip_group_check: the sim's zero-region group
                                # tracker is bank-granular and flags those
                                # disjoint groups; its per-element pending-zero
                                # numerics model still applies.
                                for k, (lhsT, rhs, tp) in enumerate(mms):
                                    nc.tensor.matmul(out_ps, lhsT, rhs,
                                                     start=(k == 0),
                                                     stop=(k == len(mms) - 1),
                                                     tile_position=tp,
                                                     skip_group_check=True)
                            for st in block_stripes:
                                used = STRIPE_USED[st]
                                nc.vector.tensor_copy(os_[st][0:used, :],
                                                      ps[st][0:used, :])
                        for b, (bs, bz) in enumerate(BINS):
                            st, off = PLACE[b]
                            eng = (nc.sync if (bz < P and off > 0)
                                   else (nc.scalar if b % 2 == 0 else nc.sync))
                            eng.dma_start(
                                o_d.ap()[bs:bs + bz, n0:n0 + NC_CHUNK],
                                os_[st][off:off + bz, :])
        nc.compile()
        return nc

    if mode == "bf16x3":
        xh_d = nc.dram_tensor("xhi", (N_SAMPLES, FREE_PER_CORE), bf16, kind="ExternalInput")
        xl_d = nc.dram_tensor("xlo", (N_SAMPLES, FREE_PER_CORE), bf16, kind="ExternalInput")
        w_d = nc.dram_tensor("wpack", (P, 2 * len(TILES) * P), bf16, kind="ExternalInput")
        xh_ap = xh_d.ap().rearrange("(t p) n -> p t n", p=P)
        xl_ap = xl_d.ap().rearrange("(t p) n -> p t n", p=P)
        w_free = 2 * len(TILES) * P
        mdt = bf16
    else:
        mdt = {"f32": f32, "f32r": mybir.dt.float32r}[mode]
        x_d = nc.dram_tensor("x", (N_SAMPLES, FREE_PER_CORE), mdt, kind="ExternalInput")
        w_d = nc.dram_tensor("wpack", (P, len(TILES) * P), mdt, kind="ExternalInput")
        x_ap = x_d.ap().rearrange("(t p) n -> p t n", p=P)
        w_free = len(TILES) * P

    with tile.TileContext(nc) as tc:
        with (
            tc.tile_pool(name="wpool", bufs=1) as wpool,
            tc.tile_pool(name="xpool", bufs=2) as xpool,
            tc.tile_pool(name="opool", bufs=2) as opool,
            tc.tile_pool(name="psum", bufs=6, space="PSUM") as psum_pool,
        ):
            w_sb = wpool.tile([P, w_free], mdt)
            nc.sync.dma_start(w_sb[:], w_d.ap())

            def wtile(t):
                return w_sb[:, t * P:(t + 1) * P]

            for _rep in range(reps):
                for c in range(N_CHUNKS):
                    n0 = c * NC_CHUNK
                    if mode == "bf16x3":
                        xh_sb = xpool.tile([P, NT, NC_CHUNK], bf16, tag="xh")
                        xl_sb = xpool.tile([P, NT, NC_CHUNK], bf16, tag="xl")
                        nc.sync.dma_start(xh_sb[:], xh_ap[:, :, n0:n0 + NC_CHUNK])
                        nc.sync.dma_start(xl_sb[:], xl_ap[:, :, n0:n0 + NC_CHUNK])
                    else:
                        x_sb = xpool.tile([P, NT, NC_CHUNK], mdt)
                        nc.sync.dma_start(x_sb[:], x_ap[:, :, n0:n0 + NC_CHUNK])
                    o_sb = opool.tile([P, NT, NC_CHUNK], f32)
                    for i in range(NT):
                        ps = psum_pool.tile([P, NC_CHUNK], f32)
                        js = CONTRIB[i]
                        if mode == "bf16x3":
                            mms = []
                            for (j, t) in js:
                                mms.append((2 * t, xh_sb[:, j, :]))      # Wh @ Xh
                                mms.append((2 * t, xl_sb[:, j, :]))      # Wh @ Xl
                                mms.append((2 * t + 1, xh_sb[:, j, :]))  # Wl @ Xh
                        else:
                            mms = [(t, x_sb[:, j, :]) for (j, t) in js]
                        for k, (wt, rhs) in enumerate(mms):
                            nc.tensor.matmul(
                                ps[:], wtile(wt), rhs,
                                start=(k == 0), stop=(k == len(mms) - 1),
                            )
                        nc.vector.tensor_copy(o_sb[:, i, :], ps[:])
                    nc.scalar.dma_start(o_ap[:, :, n0:n0 + NC_CHUNK], o_sb[:])
    nc.compile()
    return nc


_NC = None


def _get_nc():
    global _NC
    if _NC is None:
        _NC = build_program()
    return _NC


def make_in_maps(inputs, mode=MODE):
    x = np.asarray(inputs["x"], dtype=np.float32)
    mats = [np.asarray(inputs[f"mat{i}"], dtype=np.float32) for i in range(15)]
    xf = x.reshape(N_SAMPLES, FREE)
    in_maps = []
    if mode == "packed":
        w = pack_weights_packed(mats)
        xh, xl = split_x(xf)
        x2 = np.stack([xh, xl], axis=1)  # (2048, 2, 10752)
        for c in range(N_CORES):
            sl = slice(c * FREE_PER_CORE, (c + 1) * FREE_PER_CORE)
            in_maps.append({
                "x2": np.ascontiguousarray(x2[:, :, sl]),
                "wpack": w,
            })
    elif mode == "bf16x3":
        w = pack_weights_bf16(mats)
        xh, xl = split_x(xf)
        for c in range(N_CORES):
            sl = slice(c * FREE_PER_CORE, (c + 1) * FREE_PER_CORE)
            in_maps.append({
                "xhi": np.ascontiguousarray(xh[:, sl]),
                "xlo": np.ascontiguousarray(xl[:, sl]),
                "wpack": w,
            })
    else:
        w = pack_weights_f32(mats)
        for c in range(N_CORES):
            sl = slice(c * FREE_PER_CORE, (c + 1) * FREE_PER_CORE)
            in_maps.append({"x": np.ascontiguousarray(xf[:, sl]), "wpack": w})
    return in_maps


def assemble(results):
    outs = [results[c]["out"] for c in range(N_CORES)]
    full = np.concatenate(outs, axis=1)
    return full.reshape(N_SAMPLES, LENGTH, ALPHABET)


def run(inputs, nc=None, mode=MODE, **kw):
    res = bass_utils.run_bass_kernel_spmd(
        nc if nc is not None else _get_nc(),
        make_in_maps(inputs, mode=mode), core_ids=list(range(N_CORES)), **kw,
    )
    return assemble(res.results), res


def kernel(**inputs):
    out, _ = run(inputs)
    return out



# revision 3
# speedup vs baseline: 7.9338x; 7.9338x over previous
"""Block-diagonal grouped matmul (nn_MatrixApply) on 8 TRN2 NeuronCores.

Math: out[s:s+g] = mat_i @ x[s:s+g] for 15 consecutive sample groups.
Equivalently out = BD @ x_flat with BD = blockdiag(mat_0..mat_14) (2048x2048)
and x_flat = x.reshape(2048, 512*21).

Sharding: sequence-parallel. The free dim L*A = 10752 is split into 8
contiguous chunks of 1344 (= 64*21, so each core owns x[:, 64c:64(c+1), :]).
Mats are replicated. No collectives; host concatenates the slices.

Group-aligned bin/stripe structure: the 2048-sample dim is cut at group
boundaries into bins <= 128 (big groups into 128-chunks + remainder,
consecutive small groups merged while <= 128). Bins are placed into
128-partition SBUF "stripes" at 32-aligned offsets satisfying matmul
tile_position legality (size>64 -> off 0, 32<size<=64 -> off in {0,64},
size<=32 -> off in {0,32,64,96}). A matmul tile exists per (M-bin, K-bin)
pair intersected by a group: 40 tiles.

Compute modes:
  hi1 (default) - x, w, out all bf16; ONE matmul per tile (rel err ~4e-3,
      tolerance is 2e-2). Host pre-packs x into the stripe layout so each
      chunk loads with a single large DMA; output is written bf16 in stripe
      layout with one DMA per chunk and unpacked/upcast on the host.
      PSUM evacuation alternates between DVE and ACT.
  packed - previous bf16x3 scheme (hi/lo split, 3 matmuls per tile),
      per-bin DMAs. ~4e-6 rel err, ~3x the PE work.
"""

import numpy as np
import ml_dtypes

import concourse.bacc as bacc
import concourse.bass as bass
import concourse.mybir as mybir
import concourse.tile as tile
from concourse import bass_utils

BF16 = ml_dtypes.bfloat16

GROUP_SIZES = (64, 128, 256, 96, 160, 224, 192, 288, 320, 112, 80, 48, 32, 16, 32)
LENGTH = 512
ALPHABET = 21
N_SAMPLES = 2048
N_CORES = 8
FREE = LENGTH * ALPHABET            # 10752
FREE_PER_CORE = FREE // N_CORES     # 1344
P = 128
NC_CHUNK = 448                      # free-dim tile per matmul (<=512 f32 PSUM)
N_CHUNKS = FREE_PER_CORE // NC_CHUNK  # 3

MODE = "hi1"


def _make_bins():
    bins = []
    s = 0
    pend = None  # [start, size]
    for g in GROUP_SIZES:
        if g > P:
            if pend is not None:
                bins.append(tuple(pend)); pend = None
            o = 0
            while o < g:
                c = min(P, g - o)
                bins.append((s + o, c)); o += c
        elif pend is None:
            pend = [s, g]
        elif pend[1] + g <= P:
            pend[1] += g
        else:
            bins.append(tuple(pend)); pend = [s, g]
        s += g
    if pend is not None:
        bins.append(tuple(pend))
    return bins


BINS = _make_bins()          # 20 x (start, size)


def _legal_offsets(size):
    if size > 64:
        return [0]
    if size > 32:
        return [0, 64]
    return [0, 32, 64, 96]


def _place_bins():
    place = {}
    stripes = []
    for i in sorted(range(len(BINS)), key=lambda i: -BINS[i][1]):
        sz = BINS[i][1]
        done = False
        for si, st in enumerate(stripes):
            for off in _legal_offsets(sz):
                if all(off + sz <= o2 or off >= o2 + s2 for (o2, s2) in st):
                    st.append((off, sz)); place[i] = (si, off); done = True
                    break
            if done:
                break
        if not done:
            stripes.append([(0, sz)]); place[i] = (len(stripes) - 1, 0)
    used = [max(o + s for (o, s) in st) for st in stripes]
    return place, used


PLACE, STRIPE_USED = _place_bins()   # bin -> (stripe, offset); per-stripe rows used
N_STRIPES = len(STRIPE_USED)         # 17


def _make_ptiles(dup):
    """Distinct (mbin, kbin) matmul tiles, grouped per M-stripe, full-K first.

    dup=2 packs hi+lo weight planes side by side (mode packed); dup=1 packs
    only the hi plane (mode hi1)."""
    s = 0
    pairs = set()
    for g in GROUP_SIZES:
        bs = [i for i, (o, z) in enumerate(BINS) if o < s + g and o + z > s]
        for mb in bs:
            for kb in bs:
                pairs.add((mb, kb))
        s += g
    woff = {}
    off = 0
    for (mb, kb) in sorted(pairs):
        woff[(mb, kb)] = off
        off += dup * BINS[mb][1]
    # per-stripe list, K=128 tiles first (guarantees the start=True matmul
    # occupies all PE rows, so no later matmul of the same accumulation
    # group can run concurrently with the bank clear)
    per_stripe = [[] for _ in range(N_STRIPES)]
    for (mb, kb) in sorted(pairs, key=lambda p: (PLACE[p[0]][0], -BINS[p[1]][1])):
        per_stripe[PLACE[mb][0]].append((mb, kb))
    return per_stripe, woff, off


PTILES_PER_STRIPE, WOFF2, W_FREE2 = _make_ptiles(2)  # mode packed; W_FREE2 = 8192
_, WOFF1, W_FREE1 = _make_ptiles(1)                  # mode hi1;    W_FREE1 = 4096


# Emission blocks of M-bins. Bins sharing a stripe stay in one block; pairs
# are chosen so their trailing thin-K matmuls occupy disjoint 32-row groups
# of the PE array (tile_position row concurrency).
BIN_BLOCKS = [[1], [2], [3], [7, 11], [14, 9], [15, 12], [16, 5],
              [4, 6], [8, 13], [0, 10], [17], [18], [19]]


def _tiles_of_mbin(mb):
    """(mb, kb) tiles of M-bin mb, full-K first (start=True safety)."""
    st = PLACE[mb][0]
    return [t for t in PTILES_PER_STRIPE[st] if t[0] == mb]


def _dma_order():
    """K-bins in first-use order of the matmul blocks (mode packed)."""
    order = []
    for block in BIN_BLOCKS:
        for mb in block:
            for (_, kb) in _tiles_of_mbin(mb):
                if kb not in order:
                    order.append(kb)
    for b in range(len(BINS)):
        if b not in order:
            order.append(b)
    return order


DMA_ORDER = _dma_order()


def _bd(mats):
    bd = np.zeros((N_SAMPLES, N_SAMPLES), dtype=np.float32)
    start = 0
    for m in mats:
        g = m.shape[0]
        bd[start:start + g, start:start + g] = m
        start += g
    return bd


def split_x(xf):
    """f32 (n, m) -> bf16 hi, lo."""
    hi = xf.astype(BF16)
    lo = (xf - hi.astype(np.float32)).astype(BF16)
    return hi, lo


def pack_weights_packed(mats):
    """(128, W_FREE2) bf16 for mode 'packed': per (mbin,kbin) tile, the
    transposed BD block sits at partitions [k_off, k_off+ksz), free
    [woff, woff+msz) (hi) and [woff+msz, woff+2*msz) (lo)."""
    bd = _bd(mats)
    w = np.zeros((P, W_FREE2), dtype=BF16)
    for per in PTILES_PER_STRIPE:
        for (mb, kb) in per:
            (ms, mz), (ks, kz) = BINS[mb], BINS[kb]
            ko = PLACE[kb][1]
            blkT = bd[ms:ms + mz, ks:ks + kz].T  # (kz, mz)
            hi = blkT.astype(BF16)
            lo = (blkT - hi.astype(np.float32)).astype(BF16)
            o = WOFF2[(mb, kb)]
            w[ko:ko + kz, o:o + mz] = hi
            w[ko:ko + kz, o + mz:o + 2 * mz] = lo
    return w


def pack_weights_hi1(mats):
    """(128, W_FREE1) bf16 for mode 'hi1': hi plane only."""
    bd = _bd(mats)
    w = np.zeros((P, W_FREE1), dtype=BF16)
    for per in PTILES_PER_STRIPE:
        for (mb, kb) in per:
            (ms, mz), (ks, kz) = BINS[mb], BINS[kb]
            ko = PLACE[kb][1]
            blkT = bd[ms:ms + mz, ks:ks + kz].T  # (kz, mz)
            o = WOFF1[(mb, kb)]
            w[ko:ko + kz, o:o + mz] = blkT.astype(BF16)
    return w


def build_program(reps=1, mode=MODE):
    """Build the per-core Bass program.

    reps > 1 repeats the whole streaming kernel body (for wall-clock
    benchmarking via T(reps) differencing — no NTFF profiling under axon).
    """
    nc = bacc.Bacc("TRN2", target_bir_lowering=False, debug=False)
    f32 = mybir.dt.float32
    bf16 = mybir.dt.bfloat16

    if mode == "hi1":
        x_d = nc.dram_tensor("xs", (P, N_CHUNKS, N_STRIPES, NC_CHUNK), bf16,
                             kind="ExternalInput")
        w_d = nc.dram_tensor("wpack", (P, W_FREE1), bf16, kind="ExternalInput")
        o_d = nc.dram_tensor("out", (P, N_CHUNKS, N_STRIPES, NC_CHUNK), bf16,
                             kind="ExternalOutput")
        with tile.TileContext(nc) as tc:
            with (
                tc.tile_pool(name="wpool", bufs=1) as wpool,
                tc.tile_pool(name="xpool", bufs=3) as xpool,
                tc.tile_pool(name="opool", bufs=3) as opool,
                tc.tile_pool(name="psum", bufs=8, space="PSUM") as psum_pool,
            ):
                w_sb = wpool.tile([P, W_FREE1], bf16)
                nc.sync.dma_start(w_sb[:], w_d.ap())
                for _rep in range(reps):
                    for c in range(N_CHUNKS):
                        xt = xpool.tile([P, N_STRIPES, NC_CHUNK], bf16, tag="x")
                        nc.sync.dma_start(xt[:], x_d.ap()[:, c])
                        ot = opool.tile([P, N_STRIPES, NC_CHUNK], bf16, tag="o")
                        ncopy = 0
                        for block in BIN_BLOCKS:
                            block_stripes = []
                            for mb in block:
                                st = PLACE[mb][0]
                                if st not in block_stripes:
                                    block_stripes.append(st)
                            ps = {}
                            for st in block_stripes:
                                ps_tile = psum_pool.tile([P, NC_CHUNK], f32, tag="ps")
                                ps[st] = ps_tile
                            for mb in block:
                                mz = BINS[mb][1]
                                mo = PLACE[mb][1]
                                out_ps = ps[PLACE[mb][0]][mo:mo + mz, :]
                                mms = []
                                for (mb2, kb) in _tiles_of_mbin(mb):
                                    kz = BINS[kb][1]
                                    kst, ko = PLACE[kb]
                                    o = WOFF1[(mb, kb)]
                                    mms.append((w_sb[ko:ko + kz, o:o + mz],
                                                xt[ko:ko + kz, kst, :],
                                                (ko, mo)))
                                # one accumulation group per M-bin (start/stop
                                # clears are per-partition; bins sharing a psum
                                # bank at disjoint partitions are safe, hence
                                # skip_group_check).
                                for k, (lhsT, rhs, tp) in enumerate(mms):
                                    nc.tensor.matmul(out_ps, lhsT, rhs,
                                                     start=(k == 0),
                                                     stop=(k == len(mms) - 1),
                                                     tile_position=tp,
                                                     skip_group_check=True)
                            for st in block_stripes:
                                used = STRIPE_USED[st]
                                if ncopy % 2 == 0:
                                    nc.vector.tensor_copy(ot[0:used, st, :],
                                                          ps[st][0:used, :])
                                else:
                                    nc.scalar.copy(ot[0:used, st, :],
                                                   ps[st][0:used, :])
                                ncopy += 1
                        nc.scalar.dma_start(o_d.ap()[:, c], ot[:])
        nc.compile()
        return nc

    if mode == "packed":
        o_d = nc.dram_tensor("out", (N_SAMPLES, FREE_PER_CORE), f32,
                             kind="ExternalOutput")
        x2_d = nc.dram_tensor("x2", (N_SAMPLES, 2, FREE_PER_CORE), bf16,
                              kind="ExternalInput")
        w_d = nc.dram_tensor("wpack", (P, W_FREE2), bf16, kind="ExternalInput")
        with tile.TileContext(nc) as tc:
            with (
                tc.tile_pool(name="wpool", bufs=1) as wpool,
                tc.tile_pool(name="xpool", bufs=2 * N_STRIPES) as xpool,
                tc.tile_pool(name="opool", bufs=2 * N_STRIPES) as opool,
                tc.tile_pool(name="psum", bufs=8, space="PSUM") as psum_pool,
            ):
                w_sb = wpool.tile([P, W_FREE2], bf16)
                nc.sync.dma_start(w_sb[:], w_d.ap())
                for _rep in range(reps):
                    for c in range(N_CHUNKS):
                        n0 = c * NC_CHUNK
                        xs = []
                        for st in range(N_STRIPES):
                            xt = xpool.tile([P, 2, NC_CHUNK], bf16, tag="x2")
                            xs.append(xt)
                        for i, b in enumerate(DMA_ORDER):
                            bs, bz = BINS[b]
                            st, off = PLACE[b]
                            eng = (nc.scalar if (bz < P and off > 0)
                                   else (nc.sync if i % 2 == 0 else nc.scalar))
                            eng.dma_start(
                                xs[st][off:off + bz, :, :],
                                x2_d.ap()[bs:bs + bz, :, n0:n0 + NC_CHUNK])
                        os_ = []
                        for st in range(N_STRIPES):
                            ot = opool.tile([P, NC_CHUNK], f32, tag="o")
                            os_.append(ot)
                        for block in BIN_BLOCKS:
                            block_stripes = []
                            for mb in block:
                                st = PLACE[mb][0]
                                if st not in block_stripes:
                                    block_stripes.append(st)
                            ps = {}
                            for st in block_stripes:
                                ps_tile = psum_pool.tile([P, NC_CHUNK], f32, tag="ps")
                                ps[st] = ps_tile
                            for mb in block:
                                mz = BINS[mb][1]
                                mo = PLACE[mb][1]
                                out_ps = ps[PLACE[mb][0]][mo:mo + mz, :]
                                mms = []
                                for (mb2, kb) in _tiles_of_mbin(mb):
                                    kz = BINS[kb][1]
                                    kst, ko = PLACE[kb]
                                    o = WOFF2[(mb, kb)]
                                    wh = w_sb[ko:ko + kz, o:o + mz]
                                    wl = w_sb[ko:ko + kz, o + mz:o + 2 * mz]
                                    rh = xs[kst][ko:ko + kz, 0, :]
                                    rl = xs[kst][ko:ko + kz, 1, :]
                                    mms.append((wh, rh, (ko, mo)))
                                    mms.append((wh, rl, (ko, mo)))
                                    mms.append((wl, rh, (ko, mo)))
                                for k, (lhsT, rhs, tp) in enumerate(mms):
                                    nc.tensor.matmul(out_ps, lhsT, rhs,
                                                     start=(k == 0),
                                                     stop=(k == len(mms) - 1),
                                                     tile_position=tp,
                                                     skip_group_check=True)
                            for st in block_stripes:
                                used = STRIPE_USED[st]
                                nc.vector.tensor_copy(os_[st][0:used, :],
                                                      ps[st][0:used, :])
                        for b, (bs, bz) in enumerate(BINS):
                            st, off = PLACE[b]
                            eng = (nc.sync if (bz < P and off > 0)
                                   else (nc.scalar if b % 2 == 0 else nc.sync))
                            eng.dma_start(
                                o_d.ap()[bs:bs + bz, n0:n0 + NC_CHUNK],
                                os_[st][off:off + bz, :])
        nc.compile()
        return nc

    raise ValueError(mode)


_NC = None


def _get_nc():
    global _NC
    if _NC is None:
        _NC = build_program()
    return _NC


def make_in_maps(inputs, mode=MODE):
    x = np.asarray(inputs["x"], dtype=np.float32)
    mats = [np.asarray(inputs[f"mat{i}"], dtype=np.float32) for i in range(15)]
    xf = x.reshape(N_SAMPLES, FREE)
    in_maps = []
    if mode == "hi1":
        w = pack_weights_hi1(mats)
        xh = xf.astype(BF16)
        xs = np.zeros((P, N_STRIPES, FREE), dtype=BF16)
        for b, (bs, bz) in enumerate(BINS):
            st, off = PLACE[b]
            xs[off:off + bz, st, :] = xh[bs:bs + bz, :]
        for c in range(N_CORES):
            sl = xs[:, :, c * FREE_PER_CORE:(c + 1) * FREE_PER_CORE]
            xdev = np.ascontiguousarray(
                sl.reshape(P, N_STRIPES, N_CHUNKS, NC_CHUNK)
                .transpose(0, 2, 1, 3))
            in_maps.append({"xs": xdev, "wpack": w})
    elif mode == "packed":
        w = pack_weights_packed(mats)
        xh, xl = split_x(xf)
        x2 = np.stack([xh, xl], axis=1)  # (2048, 2, 10752)
        for c in range(N_CORES):
            sl = slice(c * FREE_PER_CORE, (c + 1) * FREE_PER_CORE)
            in_maps.append({
                "x2": np.ascontiguousarray(x2[:, :, sl]),
                "wpack": w,
            })
    else:
        raise ValueError(mode)
    return in_maps


def assemble(results, mode=MODE):
    if mode == "hi1":
        full = np.empty((N_SAMPLES, FREE), dtype=np.float32)
        for c in range(N_CORES):
            o = np.asarray(results[c]["out"])  # (128, 3, 17, 448) bf16
            o = (o.transpose(0, 2, 1, 3)
                 .reshape(P, N_STRIPES, FREE_PER_CORE)
                 .astype(np.float32))
            sl = slice(c * FREE_PER_CORE, (c + 1) * FREE_PER_CORE)
            for b, (bs, bz) in enumerate(BINS):
                st, off = PLACE[b]
                full[bs:bs + bz, sl] = o[off:off + bz, st]
        return full.reshape(N_SAMPLES, LENGTH, ALPHABET)
    outs = [results[c]["out"] for c in range(N_CORES)]
    full = np.concatenate(outs, axis=1)
    return full.reshape(N_SAMPLES, LENGTH, ALPHABET)


def run(inputs, nc=None, mode=MODE, **kw):
    res = bass_utils.run_bass_kernel_spmd(
        nc if nc is not None else _get_nc(),
        make_in_maps(inputs, mode=mode), core_ids=list(range(N_CORES)), **kw,
    )
    return assemble(res.results, mode=mode), res


def kernel(**inputs):
    out, _ = run(inputs)
    return out
